# revision 28
# baseline (speedup 1.0000x reference)
"""GAT (2-layer, PyG GATConv) Trainium2 Bass kernel, 8-core SPMD.

Strategy:
  - Nodes sharded 8 ways by range (6250/core, padded to 6272).
  - Edges (with self-loops) sorted by dst; each core handles edges whose dst
    is in its shard; per 128-dst-node block, edges are gathered (dma_gather
    Ant instruction) and scatter-added into PSUM via one-hot matmuls.
  - Layer-1 node projection (table1 = [h | a_s | a_d]) is computed
    replicated on all cores (x @ [W1|Vsrc|Vdst]); no collective needed.
  - Between layers one AllGather assembles the full layer-2 gather table
    (h2 bf16 + a_s2 f32) from per-core shards.
  - Softmax max-subtraction is skipped (exact same math, e is small).
"""
import sys
import os

sys.path.insert(0, "/opt/trn_rl_repo")

import numpy as np

import concourse.bass as bass
import concourse.bacc as bacc
import concourse.mybir as mybir
import concourse.tile as tile
from concourse.bass_utils import run_bass_kernel_spmd
from concourse._compat import cdiv

# ---------------- problem constants (hardcoded per spec) ----------------
N = 50000
E0 = 800000
IN_CH = 128
HID = 64
HEADS = 4
OUT_CH = 16
NEG = 0.2

NCORES = 8
NS = 6250            # nodes per core
NSP = 6272           # padded shard rows (49*128)
NB = 49              # dst blocks per core
P = 128

SPLIT1 = 25000       # src-space lo/hi split for table1 (int16 gather idx)
SPLIT2 = 25088       # = 4*6272, same edge partition in table2 row space

T1_ROWS = 50048      # 391*128, last tile padded
T1_COLS = 320        # f32: [h 0:256 | a_s 256:260 | a_d 260:264 | pad] (1280B)
T2_COLS = 128        # bf16: [h2 0:64 | a_s2(f32) 64:66 | pad] (256B)
ATT_COLS = 64        # f32 rows (256B): att1_local=[a_d(4)], att2_local=[a_d2(1)]

F32 = mybir.dt.float32
BF16 = mybir.dt.bfloat16
I16 = mybir.dt.int16

_EXEC_INFO = {}      # test.py reads this


# ---------------- CPU preprocessing ----------------

def _wrap_idx(vals):
    """int16 values -> [128, len/16] gather-index layout (16-wrap, x8 replicate)."""
    k = len(vals) // 16
    arr = np.asarray(vals, dtype=np.int16).reshape(k, 16).T  # [16, k]
    return np.tile(arr, (8, 1))  # [128, k]


def preprocess(edge_index):
    ei = np.asarray(edge_index)
    src = np.concatenate([ei[0], np.arange(N, dtype=ei.dtype)]).astype(np.int64)
    dst = np.concatenate([ei[1], np.arange(N, dtype=ei.dtype)]).astype(np.int64)
    order = np.argsort(dst, kind="stable")
    srcs = src[order].astype(np.int32)
    dsts = dst[order].astype(np.int32)

    # block boundaries: core c, block b covers nodes [c*NS + b*128, ...)
    lo_nodes = np.array([c * NS + b * 128 for c in range(NCORES) for b in range(NB)])
    hi_nodes = np.array(
        [min(c * NS + (b + 1) * 128, (c + 1) * NS) for c in range(NCORES) for b in range(NB)]
    )
    starts = np.searchsorted(dsts, lo_nodes)
    ends = np.searchsorted(dsts, hi_nodes)

    # per (c,b): split by src < SPLIT1 (same partition works for layer 2)
    n_lo = np.zeros((NCORES, NB), np.int64)
    n_hi = np.zeros((NCORES, NB), np.int64)
    segs = {}
    for c in range(NCORES):
        for b in range(NB):
            i = c * NB + b
            s, e = starts[i], ends[i]
            ss, dd = srcs[s:e], dsts[s:e]
            m = ss < SPLIT1
            segs[(c, b)] = (ss[m], dd[m], ss[~m], dd[~m])
            n_lo[c, b] = m.sum()
            n_hi[c, b] = (~m).sum()

    TL = [int(cdiv(int(n_lo[:, b].max()), P)) for b in range(NB)]
    TH = [int(cdiv(int(n_hi[:, b].max()), P)) for b in range(NB)]
    TL = [max(t, 1) for t in TL]
    TH = [max(t, 1) for t in TH]
    TB = [TL[b] + TH[b] for b in range(NB)]

    per_core = []
    for c in range(NCORES):
        idx1_cols, idx2_cols, idxa_cols, dloc_cols = [], [], [], []
        for b in range(NB):
            slo, dlo, shi, dhi = segs[(c, b)]
            npad_lo = TL[b] * P - len(slo)
            npad_hi = TH[b] * P - len(shi)
            blk_base = c * NS + b * 128

            s_all = np.concatenate([
                slo, np.zeros(npad_lo, np.int64),
                shi, np.full(npad_hi, SPLIT1, np.int64),
            ])
            d_all = np.concatenate([
                dlo, np.full(npad_lo, -1, np.int64),
                dhi, np.full(npad_hi, -1, np.int64),
            ])
            valid = d_all >= 0
            dloc = np.where(valid, d_all - blk_base, -1).astype(np.float32)

            # main1 idx: lo: src ; hi: src - SPLIT1
            i1 = s_all.copy()
            i1[TL[b] * P:] -= SPLIT1
            idx1_cols.append(np.concatenate([
                _wrap_idx(i1[: TL[b] * P]), _wrap_idx(i1[TL[b] * P:])], axis=1))

            # main2 idx: padded table2 row; lo: row ; hi: row - SPLIT2
            row2 = (s_all // NS) * NSP + (s_all % NS)
            i2 = row2.copy()
            i2[TL[b] * P:] -= SPLIT2
            assert i2.max() < 32768 and i2.min() >= 0
            idx2_cols.append(np.concatenate([
                _wrap_idx(i2[: TL[b] * P]), _wrap_idx(i2[TL[b] * P:])], axis=1))

            # att-dst idx: local shard row of dst (0 for pads)
            ia = np.where(valid, d_all - c * NS, 0)
            idxa_cols.append(_wrap_idx(ia))

            # dst_local tile-column layout [p, t]
            dloc_cols.append(dloc.reshape(TB[b], P).T.astype(np.float32))

        per_core.append({
            "IDX1": np.concatenate(idx1_cols, axis=1),
            "IDX2": np.concatenate(idx2_cols, axis=1),
            "IDXA": np.concatenate(idxa_cols, axis=1),
            "DLOC": np.concatenate(dloc_cols, axis=1),
        })

    struct = {"TL": TL, "TH": TH, "TB": TB}
    return struct, per_core


# ---------------- device program ----------------

def build_program(struct):
    TL, TH, TB = struct["TL"], struct["TH"], struct["TB"]
    C1 = sum(TB) * 8          # idx cols for main (TL+TH per block, *8)
    CA = sum(TB) * 8
    CT = sum(TB)
    TMAX = max(TB)

    nc = bacc.Bacc("TRN2", target_bir_lowering=False, debug=False,
                   num_devices=NCORES)

    # inputs
    xT = nc.dram_tensor("xT", [P, T1_ROWS], F32, kind="ExternalInput")
    xmT = nc.dram_tensor("xmT", [P, NSP], F32, kind="ExternalInput")
    w1cat = nc.dram_tensor("w1cat", [P, 264], F32, kind="ExternalInput")
    vdst1 = nc.dram_tensor("vdst1", [P, 4], F32, kind="ExternalInput")
    w2cat = nc.dram_tensor("w2cat", [256, 66], BF16, kind="ExternalInput")
    wlin = nc.dram_tensor("wlin", [HID, OUT_CH], F32, kind="ExternalInput")
    b1rep = nc.dram_tensor("b1rep", [P, 256], F32, kind="ExternalInput")
    b2rep = nc.dram_tensor("b2rep", [P, HID], F32, kind="ExternalInput")
    blrep = nc.dram_tensor("blrep", [P, OUT_CH], F32, kind="ExternalInput")
    iota = nc.dram_tensor("iota", [P, 1, P], F32, kind="ExternalInput")
    ident = nc.dram_tensor("ident", [P, P], F32, kind="ExternalInput")
    IDX1 = nc.dram_tensor("IDX1", [P, C1], I16, kind="ExternalInput")
    IDX2 = nc.dram_tensor("IDX2", [P, C1], I16, kind="ExternalInput")
    IDXA = nc.dram_tensor("IDXA", [P, CA], I16, kind="ExternalInput")
    DLOC = nc.dram_tensor("DLOC", [P, CT], F32, kind="ExternalInput")

    out_sh = nc.dram_tensor("out", [NSP, OUT_CH], F32, kind="ExternalOutput")
    debug = int(os.environ.get("GAT_DEBUG", "0"))
    if debug:
        dbg = nc.dram_tensor("dbg", [NSP, 256], F32, kind="ExternalOutput")
    if debug >= 2:
        TB0 = TB[0]
        dbg_g = nc.dram_tensor("dbg_g", [P, TB0 * 320], F32, kind="ExternalOutput")
        dbg_ad = nc.dram_tensor("dbg_ad", [P, TB0 * 64], F32, kind="ExternalOutput")
        dbg_oh = nc.dram_tensor("dbg_oh", [P, TB0 * P], BF16, kind="ExternalOutput")
        dbg_e = nc.dram_tensor("dbg_e", [P, TB0 * 4], F32, kind="ExternalOutput")
        dbg_acc = nc.dram_tensor("dbg_acc", [P, 260], F32, kind="ExternalOutput")

    NT1 = T1_ROWS // P  # 391

    with tile.TileContext(nc) as tc:
        with (
            tc.tile_pool(name="dramp", bufs=1, space="DRAM") as dram,
            tc.tile_pool(name="const", bufs=1) as cst,
        ):
            # internal DRAM (pool tiles so Tile tracks cross-phase deps)
            t1 = dram.tile([T1_ROWS, T1_COLS], F32, name="t1")
            att1 = dram.tile([NSP, ATT_COLS], F32, name="att1")
            att2 = dram.tile([NSP, ATT_COLS], F32, name="att2")
            t2s = dram.tile([NSP, T2_COLS], BF16, name="t2s")
            t2f = dram.tile([NCORES * NSP, T2_COLS], BF16, name="t2f",
                            addr_space="Shared")
            w1_sb = cst.tile([P, 264], F32)
            nc.sync.dma_start(out=w1_sb[:], in_=w1cat[:])
            vd1_sb = cst.tile([P, 4], F32)
            nc.sync.dma_start(out=vd1_sb[:], in_=vdst1[:])
            w2a_sb = cst.tile([P, 66], BF16)
            nc.sync.dma_start(out=w2a_sb[:], in_=w2cat[0:128, :])
            w2b_sb = cst.tile([P, 66], BF16)
            nc.sync.dma_start(out=w2b_sb[:], in_=w2cat[128:256, :])
            wl_sb = cst.tile([HID, OUT_CH], F32)
            nc.sync.dma_start(out=wl_sb[:], in_=wlin[:])
            b1_sb = cst.tile([P, 256], F32)
            nc.sync.dma_start(out=b1_sb[:], in_=b1rep[:])
            b2_sb = cst.tile([P, HID], F32)
            nc.sync.dma_start(out=b2_sb[:], in_=b2rep[:])
            bl_sb = cst.tile([P, OUT_CH], F32)
            nc.sync.dma_start(out=bl_sb[:], in_=blrep[:])
            io_sb = cst.tile([P, 1, P], F32)
            nc.sync.dma_start(out=io_sb[:], in_=iota[:])
            id_sb = cst.tile([P, P], F32)
            nc.sync.dma_start(out=id_sb[:], in_=ident[:])
            x2T = cst.tile([P, 2, NSP], BF16)  # persistent layer-2 input (transposed)

            reps = int(os.environ.get("GAT_REPS", "1"))
            for _rep in range(reps):
                _build_phases(nc, tc, locals())
    nc.compile()
    return nc


def _build_phases(nc, tc, env):
    """One repetition of all compute phases (split out for GAT_REPS timing)."""
    TL, TH, TB = env["TL"], env["TH"], env["TB"]
    TMAX = env["TMAX"]
    xT, xmT = env["xT"], env["xmT"]
    w1_sb, vd1_sb = env["w1_sb"], env["vd1_sb"]
    w2a_sb, w2b_sb, wl_sb = env["w2a_sb"], env["w2b_sb"], env["wl_sb"]
    b1_sb, b2_sb, bl_sb = env["b1_sb"], env["b2_sb"], env["bl_sb"]
    io_sb, id_sb, x2T = env["io_sb"], env["id_sb"], env["x2T"]
    IDX1, IDX2, IDXA, DLOC = env["IDX1"], env["IDX2"], env["IDXA"], env["DLOC"]
    t1, att1, att2, t2s, t2f = (env["t1"], env["att1"], env["att2"],
                                env["t2s"], env["t2f"])
    out_sh = env["out_sh"]
    debug = env["debug"]
    if debug:
        dbg = env["dbg"]
    if debug >= 2:
        dbg_g, dbg_ad, dbg_oh, dbg_e, dbg_acc = (
            env["dbg_g"], env["dbg_ad"], env["dbg_oh"], env["dbg_e"],
            env["dbg_acc"])
    NT1 = env["NT1"]

    if True:
        if True:
            # ---------- phase A: build table1 (replicated) ----------
            with (
                tc.tile_pool(name="pa", bufs=3) as pa,
                tc.tile_pool(name="pa_ps", bufs=2, space="PSUM") as pa_ps,
            ):
                for nt in range(NT1):
                    xt_t = pa.tile([P, P], F32, tag="xt")
                    nc.sync.dma_start(out=xt_t[:], in_=xT[:, nt * P:(nt + 1) * P])
                    ps = pa_ps.tile([P, 264], F32)
                    nc.tensor.matmul(ps[:], lhsT=xt_t[:], rhs=w1_sb[:],
                                     start=True, stop=True)
                    row = pa.tile([P, 264], F32, tag="row")
                    if nt % 2 == 0:
                        nc.vector.tensor_copy(out=row[:], in_=ps[:])
                    else:
                        nc.scalar.copy(row[:], ps[:])
                    nc.sync.dma_start(out=t1[nt * P:(nt + 1) * P, 0:264],
                                      in_=row[:])
                # phase A-mine: att1_local = x_mine @ Vdst1
                for b in range(NB):
                    xm_t = pa.tile([P, P], F32, tag="xt")
                    nc.sync.dma_start(out=xm_t[:], in_=xmT[:, b * P:(b + 1) * P])
                    psm = pa_ps.tile([P, 264], F32)
                    nc.tensor.matmul(psm[:, 0:4], lhsT=xm_t[:], rhs=vd1_sb[:],
                                     start=True, stop=True)
                    rw4 = pa.tile([P, 4], F32, tag="rw4")
                    nc.vector.tensor_copy(out=rw4[:], in_=psm[:, 0:4])
                    nc.sync.dma_start(out=att1[b * P:(b + 1) * P, 0:4],
                                      in_=rw4[:])

            # ---------- layer 1 edge phase ----------
            def edge_layer(layer):
                if layer == 1:
                    FW = 260          # rhs width: 256 feat + 4 exp
                    NH = 4
                else:
                    FW = 65           # 64 feat + 1 exp
                    NH = 1
                with (
                    tc.tile_pool(name=f"eg{layer}", bufs=2) as eg,
                    tc.tile_pool(name=f"eg{layer}_ps", bufs=2, space="PSUM") as egp,
                    tc.tile_pool(name=f"ev{layer}", bufs=2) as ev,
                    tc.tile_pool(name=f"ev{layer}_ps", bufs=2, space="PSUM") as evp,
                ):
                    l1mode = os.environ.get("GAT_L1MODE", "full")
                    nb_lim = int(os.environ.get("GAT_NB", str(NB)))
                    off_m = 0
                    off_a = 0
                    off_t = 0
                    for b in range(min(NB, nb_lim) if layer == 1 else NB):
                        tl, th, tb = TL[b], TH[b], TB[b]
                        nv = 128 if b < NB - 1 else NS - 128 * (NB - 1)
                        # --- load idx/meta ---
                        ixm = eg.tile([P, TMAX * 8], I16, tag="ixm")
                        src_idx = IDX1 if layer == 1 else IDX2
                        nc.sync.dma_start(out=ixm[:, 0:tb * 8],
                                          in_=src_idx[:, off_m:off_m + tb * 8])
                        ixa = eg.tile([P, TMAX * 8], I16, tag="ixa")
                        nc.sync.dma_start(out=ixa[:, 0:tb * 8],
                                          in_=IDXA[:, off_a:off_a + tb * 8])
                        dlc = eg.tile([P, TMAX], F32, tag="dlc")
                        nc.sync.dma_start(out=dlc[:, 0:tb],
                                          in_=DLOC[:, off_t:off_t + tb])
                        # --- gathers (chunked: >=1280 idx per dma_gather
                        # wedges the device; use <=1024 = 8 tiles) ---
                        def gat(out_t, c0, nt, src_ap, idx_t, ic0, elem):
                            done = 0
                            while done < nt:
                                k = min(8, nt - done)
                                nc.gpsimd.dma_gather(
                                    out_t[:, c0 + done:c0 + done + k, :],
                                    src_ap,
                                    idx_t[:, ic0 + done * 8:ic0 + (done + k) * 8],
                                    k * P, k * P, elem)
                                done += k

                        if layer == 1:
                            g = eg.tile([P, TMAX, 320], F32, tag="g1")
                            gat(g, 0, tl, t1[:, 0:320], ixm, 0, 320)
                            gat(g, tl, th, t1[SPLIT1:T1_ROWS, 0:320],
                                ixm, tl * 8, 320)
                            ad = eg.tile([P, TMAX, 64], F32, tag="ad")
                            gat(ad, 0, tb, att1[:, 0:64], ixa, 0, 64)
                            a_s = g[:, 0:tb, 256:260]
                            a_d = ad[:, 0:tb, 0:4]
                            feat = lambda h: g[:, 0:tb, h * 64:(h + 1) * 64]
                        else:
                            g = eg.tile([P, TMAX, 128], BF16, tag="g2")
                            gat(g, 0, tl, t2f[0:SPLIT2, :], ixm, 0, 128)
                            gat(g, tl, th, t2f[SPLIT2:NCORES * NSP, :],
                                ixm, tl * 8, 128)
                            ad = eg.tile([P, TMAX, 64], F32, tag="ad")
                            gat(ad, 0, tb, att2[:, 0:64], ixa, 0, 64)
                            a_s = g[:, 0:tb, 64:66].bitcast(F32)
                            a_d = ad[:, 0:tb, 0:1]
                            feat = lambda h: g[:, 0:tb, 0:64]
                        if layer == 1 and l1mode == "gather":
                            cons = eg.tile([P, OUT_CH], F32, tag="cons")
                            nc.vector.tensor_copy(out=cons[:],
                                                  in_=g[:, 0, 0:OUT_CH])
                            nc.vector.tensor_tensor(out=cons[:], in0=cons[:],
                                                    in1=ad[:, 0, 0:OUT_CH],
                                                    op=mybir.AluOpType.add)
                            if b == 0:
                                nc.sync.dma_start(out=out_sh[0:P, :],
                                                  in_=cons[:])
                            off_m += tb * 8
                            off_a += tb * 8
                            off_t += tb
                            continue
                        # --- e = leaky(a_s + a_d); exp ---
                        s_t = eg.tile([P, TMAX, NH], F32, tag="s")
                        nc.vector.tensor_tensor(out=s_t[:, 0:tb, :], in0=a_s,
                                                in1=a_d, op=mybir.AluOpType.add)
                        s2_t = eg.tile([P, TMAX, NH], F32, tag="s2")
                        nc.vector.tensor_scalar_mul(s2_t[:, 0:tb, :],
                                                    s_t[:, 0:tb, :], NEG)
                        lk_t = eg.tile([P, TMAX, NH], F32, tag="lk")
                        nc.vector.tensor_tensor(out=lk_t[:, 0:tb, :],
                                                in0=s_t[:, 0:tb, :],
                                                in1=s2_t[:, 0:tb, :],
                                                op=mybir.AluOpType.max)
                        e_t = eg.tile([P, TMAX, NH], F32, tag="e")
                        nc.scalar.activation(e_t[:, 0:tb, :], lk_t[:, 0:tb, :],
                                             mybir.ActivationFunctionType.Exp)
                        # --- R = [feat*exp | exp] (bf16), onehot ---
                        r = eg.tile([P, TMAX, FW], BF16, tag="r")
                        for h in range(NH):
                            nc.vector.tensor_tensor(
                                out=r[:, 0:tb, h * 64:(h + 1) * 64],
                                in0=feat(h),
                                in1=e_t[:, 0:tb, h:h + 1].to_broadcast([P, tb, 64]),
                                op=mybir.AluOpType.mult)
                        nc.vector.tensor_copy(out=r[:, 0:tb, NH * 64:NH * 64 + NH],
                                              in_=e_t[:, 0:tb, :])
                        oh = eg.tile([P, TMAX, P], BF16, tag="oh")
                        nc.vector.tensor_tensor(
                            out=oh[:, 0:tb, :],
                            in0=io_sb[:].to_broadcast([P, tb, P]),
                            in1=dlc[:, 0:tb, None].to_broadcast([P, tb, P]),
                            op=mybir.AluOpType.is_equal)
                        # --- scatter matmuls ---
                        acc = egp.tile([P, FW], F32)
                        for t in range(tb):
                            nc.tensor.matmul(acc[:], lhsT=oh[:, t, :],
                                             rhs=r[:, t, :],
                                             start=(t == 0), stop=(t == tb - 1))
                        if layer == 1 and b == 0 and debug >= 2:
                            nc.sync.dma_start(out=dbg_g[:],
                                              in_=g[:, 0:tb, :].opt())
                            nc.sync.dma_start(out=dbg_ad[:],
                                              in_=ad[:, 0:tb, :].opt())
                            nc.sync.dma_start(out=dbg_oh[:],
                                              in_=oh[:, 0:tb, :].opt())
                            nc.sync.dma_start(out=dbg_e[:],
                                              in_=e_t[:, 0:tb, :].opt())
                            acc_cp = ev.tile([P, 260], F32, tag="acc_cp")
                            nc.vector.tensor_copy(out=acc_cp[:], in_=acc[:, 0:260])
                            nc.sync.dma_start(out=dbg_acc[:], in_=acc_cp[:])
                        # --- eviction ---
                        NF = NH * 64
                        dn = ev.tile([P, NH], F32, tag="dn")
                        nc.vector.tensor_scalar_max(dn[:], acc[:, NF:NF + NH],
                                                    1e-30)
                        rc = ev.tile([P, NH], F32, tag="rc")
                        nc.vector.reciprocal(rc[:], dn[:])
                        xo = ev.tile([P, NF], F32, tag="xo")
                        for h in range(NH):
                            nc.vector.tensor_scalar_mul(
                                xo[:, h * 64:(h + 1) * 64],
                                acc[:, h * 64:(h + 1) * 64], rc[:, h:h + 1])
                        xb = ev.tile([P, NF], F32, tag="xb")
                        nc.vector.tensor_tensor(
                            out=xb[:], in0=xo[:],
                            in1=(b1_sb[:] if layer == 1 else b2_sb[:]),
                            op=mybir.AluOpType.add)
                        # ELU: exp(min(x,0)) + (max(x,0)-1)
                        mn = ev.tile([P, NF], F32, tag="mn")
                        nc.vector.tensor_scalar_min(mn[:], xb[:], 0.0)
                        ex = ev.tile([P, NF], F32, tag="ex")
                        nc.scalar.activation(ex[:], mn[:],
                                             mybir.ActivationFunctionType.Exp)
                        px = ev.tile([P, NF], F32, tag="px")
                        nc.vector.tensor_scalar(px[:], xb[:], 0.0, -1.0,
                                                mybir.AluOpType.max,
                                                mybir.AluOpType.add)
                        xe = ev.tile([P, NF], F32, tag="xe")
                        nc.vector.tensor_tensor(out=xe[:], in0=ex[:], in1=px[:],
                                                op=mybir.AluOpType.add)
                        if layer == 1 and debug:
                            nc.sync.dma_start(
                                out=dbg[b * P:b * P + nv, :], in_=xe[0:nv, :])
                        if layer == 1:
                            # transpose into persistent x2T (bf16)
                            for k in range(2):
                                tp = evp.tile([P, P], F32, tag="tp")
                                nc.tensor.transpose(tp[:], xe[:, k * P:(k + 1) * P],
                                                    id_sb[:])
                                nc.vector.tensor_copy(
                                    out=x2T[:, k, b * P:(b + 1) * P], in_=tp[:])
                        else:
                            # final linear: out = elu(x3) @ Wlin + blin
                            tp = evp.tile([P, P], F32, tag="tp")
                            nc.tensor.transpose(tp[0:64, 0:P], xe[:, 0:64],
                                                id_sb[:])
                            x3T = ev.tile([HID, P], F32, tag="x3T")
                            nc.vector.tensor_copy(out=x3T[:], in_=tp[0:64, 0:P])
                            ops = evp.tile([P, OUT_CH], F32, tag="ops")
                            nc.tensor.matmul(ops[:], lhsT=x3T[:], rhs=wl_sb[:],
                                             start=True, stop=True)
                            ob = ev.tile([P, OUT_CH], F32, tag="ob")
                            nc.vector.tensor_tensor(out=ob[:], in0=ops[:],
                                                    in1=bl_sb[:],
                                                    op=mybir.AluOpType.add)
                            nc.sync.dma_start(
                                out=out_sh[b * P:b * P + nv, :], in_=ob[0:nv, :])
                        off_m += tb * 8
                        off_a += tb * 8
                        off_t += tb

            phases = os.environ.get("GAT_PHASES", "full")
            if phases != "a":
                edge_layer(1)

            # ---------- table2 shard build + AllGather ----------
            if phases in ("full", "a1t", "a1tc"):
                with (
                    tc.tile_pool(name="t2", bufs=3) as t2p,
                    tc.tile_pool(name="t2_ps", bufs=2, space="PSUM") as t2ps,
                ):
                    for b in range(NB):
                        ps2 = t2ps.tile([P, 66], F32)
                        nc.tensor.matmul(ps2[:], lhsT=x2T[:, 0, b * P:(b + 1) * P],
                                         rhs=w2a_sb[:], start=True, stop=False)
                        nc.tensor.matmul(ps2[:], lhsT=x2T[:, 1, b * P:(b + 1) * P],
                                         rhs=w2b_sb[:], start=False, stop=True)
                        h2 = t2p.tile([P, HID], BF16, tag="h2")
                        nc.vector.tensor_copy(out=h2[:], in_=ps2[:, 0:64])
                        av = t2p.tile([P, 2], F32, tag="av")
                        nc.vector.tensor_copy(out=av[:], in_=ps2[:, 64:66])
                        nc.sync.dma_start(out=t2s[b * P:(b + 1) * P, 0:64],
                                          in_=h2[:])
                        nc.sync.dma_start(
                            out=t2s[b * P:(b + 1) * P, 64:66].bitcast(F32),
                            in_=av[:, 0:1])
                        nc.sync.dma_start(out=att2[b * P:(b + 1) * P, 0:1],
                                          in_=av[:, 1:2])
            if phases in ("full", "a1tc"):
                nc.gpsimd.collective_compute(
                    "AllGather",
                    mybir.AluOpType.bypass,
                    replica_groups=[list(range(NCORES))],
                    ins=[t2s[:].opt()],
                    outs=[t2f[:].opt()],
                )

            # ---------- layer 2 edge phase + output ----------
            if phases == "full":
                edge_layer(2)


# ---------------- host orchestration ----------------

def _prep_weights(W1, att_src1, att_dst1, b1, W2, att_src2, att_dst2, b2,
                  Wlin, blin):
    W1 = np.asarray(W1, np.float32)
    vsrc1 = np.zeros((IN_CH, HEADS), np.float32)
    vdst1 = np.zeros((IN_CH, HEADS), np.float32)
    a_s1 = np.asarray(att_src1, np.float32)
    a_d1 = np.asarray(att_dst1, np.float32)
    for h in range(HEADS):
        vsrc1[:, h] = W1[:, h * HID:(h + 1) * HID] @ a_s1[h]
        vdst1[:, h] = W1[:, h * HID:(h + 1) * HID] @ a_d1[h]
    w1cat = np.concatenate([W1, vsrc1, vdst1], axis=1)  # [128, 264]

    W2 = np.asarray(W2, np.float32)
    vsrc2 = W2 @ np.asarray(att_src2, np.float32)[0]
    vdst2 = W2 @ np.asarray(att_dst2, np.float32)[0]
    w2cat = np.concatenate([W2, vsrc2[:, None], vdst2[:, None]], axis=1)  # [256,66]

    import ml_dtypes

    return {
        "w1cat": w1cat,
        "vdst1": vdst1,
        "w2cat": w2cat.astype(ml_dtypes.bfloat16),
        "wlin": np.asarray(Wlin, np.float32),
        "b1rep": np.tile(np.asarray(b1, np.float32)[None, :], (P, 1)),
        "b2rep": np.tile(np.asarray(b2, np.float32)[None, :], (P, 1)),
        "blrep": np.tile(np.asarray(blin, np.float32)[None, :], (P, 1)),
        "iota": np.tile(np.arange(P, dtype=np.float32)[None, None, :], (P, 1, 1)),
        "ident": np.eye(P, dtype=np.float32),
    }


_CACHE = {}


def time_kernel(nc, in_maps, iters=5):
    """Steady-state device execution time: jit once, device-put inputs,
    time blocked executions (no host->device transfer in the loop)."""
    import time as _time
    import jax
    from jax.sharding import Mesh, PartitionSpec, NamedSharding
    from jax.experimental.shard_map import shard_map
    from concourse import bass2jax as b2j
    import concourse.mybir as mb

    b2j.install_neuronx_cc_hook()
    n_cores = len(in_maps)
    partition_name = (nc.partition_id_tensor.name
                      if nc.partition_id_tensor else None)
    in_names, out_names, out_avals, zero_outs = [], [], [], []
    for alloc in nc.m.functions[0].allocations:
        if not isinstance(alloc, mb.MemoryLocationSet):
            continue
        name = alloc.memorylocations[0].name
        if alloc.kind == "ExternalInput":
            if name != partition_name:
                in_names.append(name)
        elif alloc.kind == "ExternalOutput":
            out_avals.append(jax.core.ShapedArray(
                tuple(alloc.tensor_shape), mb.dt.np(alloc.dtype)))
            out_names.append(name)
            zero_outs.append(np.zeros(alloc.tensor_shape,
                                      mb.dt.np(alloc.dtype)))
    n_params = len(in_names)
    all_in_names = list(in_names) + list(out_names)
    if partition_name is not None:
        all_in_names.append(partition_name)

    def _body(*args):
        operands = list(args)
        if partition_name is not None:
            operands.append(b2j.partition_id_tensor())
        return tuple(b2j._bass_exec_p.bind(
            *operands,
            out_avals=tuple(out_avals),
            in_names=tuple(all_in_names),
            out_names=tuple(out_names),
            lowering_input_output_aliases=(),
            sim_require_finite=True,
            sim_require_nnan=True,
            nc=nc,
        ))

    devices = jax.devices()[:n_cores]
    mesh = Mesh(np.asarray(devices), ("core",))
    nouts = len(out_names)
    in_specs = (PartitionSpec("core"),) * (n_params + nouts)
    out_specs = (PartitionSpec("core"),) * nouts
    fn = jax.jit(shard_map(_body, mesh=mesh, in_specs=in_specs,
                           out_specs=out_specs, check_rep=False),
                 keep_unused=True)
    sh = NamedSharding(mesh, PartitionSpec("core"))
    concat_in = [
        jax.device_put(np.concatenate(
            [np.asarray(in_maps[c][nm]) for c in range(n_cores)], axis=0), sh)
        for nm in in_names
    ]
    concat_zero = [
        jax.device_put(np.zeros((n_cores * z.shape[0], *z.shape[1:]), z.dtype),
                       sh)
        for z in zero_outs
    ]
    outs = fn(*concat_in, *concat_zero)  # warmup / compile
    jax.block_until_ready(outs)
    ts = []
    for _ in range(iters):
        t0 = _time.perf_counter()
        outs = fn(*concat_in, *concat_zero)
        jax.block_until_ready(outs)
        ts.append(_time.perf_counter() - t0)
    return min(ts), ts


def kernel(x, edge_index, edge_attr, W1, att_src1, att_dst1, b1,
           W2, att_src2, att_dst2, b2, Wlin, blin):
    x = np.asarray(x, np.float32)
    struct, per_core = preprocess(edge_index)

    key = (tuple(struct["TL"]), tuple(struct["TH"]))
    if key not in _CACHE:
        _CACHE[key] = build_program(struct)
    nc = _CACHE[key]

    wts = _prep_weights(W1, att_src1, att_dst1, b1, W2, att_src2, att_dst2,
                        b2, Wlin, blin)
    xT = np.zeros((P, T1_ROWS), np.float32)
    xT[:, :N] = x.T
    in_maps = []
    for c in range(NCORES):
        xmT = np.zeros((P, NSP), np.float32)
        xmT[:, :NS] = x[c * NS:(c + 1) * NS].T
        m = {"xT": xT, "xmT": xmT}
        m.update(wts)
        m.update(per_core[c])
        in_maps.append(m)

    trace = bool(int(os.environ.get("GAT_TRACE", "0")))
    res = run_bass_kernel_spmd(nc, in_maps, core_ids=list(range(NCORES)),
                               trace=trace)
    _EXEC_INFO["exec_time_ns"] = res.exec_time_ns
    _EXEC_INFO["profile_json"] = res.profile_json

    if bool(int(os.environ.get("GAT_TIME", "0"))):
        best, ts = time_kernel(nc, in_maps)
        _EXEC_INFO["exec_time_ns"] = int(best * 1e9)
        _EXEC_INFO["times"] = ts

    if "dbg" in res.results[0]:
        _EXEC_INFO["dbg"] = np.concatenate(
            [res.results[c]["dbg"][0:NS] for c in range(NCORES)], axis=0)
    _EXEC_INFO["core0"] = res.results[0]

    out = np.concatenate([res.results[c]["out"][0:NS] for c in range(NCORES)],
                         axis=0)
    return out.astype(np.float32)


# revision 39
# speedup vs baseline: 10.0883x; 10.0883x over previous
"""GAT (2-layer, PyG GATConv) Trainium2 Bass kernel, 8-core SPMD.

Strategy:
  - Nodes sharded 8 ways by range (6250/core, padded to 6272).
  - Edges (with self-loops) sorted by dst; each core handles edges whose dst
    is in its shard; per 128-dst-node block, edges are gathered (dma_gather
    Ant instruction) and scatter-added into PSUM via one-hot matmuls.
  - Layer-1 node projection (table1 = [h | a_s | a_d]) is computed
    replicated on all cores (x @ [W1|Vsrc|Vdst]); no collective needed.
  - Between layers one AllGather assembles the full layer-2 gather table
    (h2 bf16 + a_s2 f32) from per-core shards.
  - Softmax max-subtraction is skipped (exact same math, e is small).
"""
import sys
import os

sys.path.insert(0, "/opt/trn_rl_repo")

import numpy as np
import ml_dtypes

import concourse.bass as bass
import concourse.bacc as bacc
import concourse.mybir as mybir
import concourse.tile as tile
from concourse.bass_utils import run_bass_kernel_spmd
from concourse._compat import cdiv

# ---------------- problem constants (hardcoded per spec) ----------------
N = 50000
E0 = 800000
IN_CH = 128
HID = 64
HEADS = 4
OUT_CH = 16
NEG = 0.2

NCORES = 8
NS = 6250            # nodes per core
NSP = 6272           # padded shard rows (49*128)
NB = 49              # dst blocks per core
P = 128

SPLIT1 = 25000       # src-space lo/hi split for table1 (int16 gather idx)
SPLIT2 = 25088       # = 4*6272, same edge partition in table2 row space

T1_ROWS = 50048      # 391*128, last tile padded
T1_COLS = 384        # bf16: [h 0:256 | a_s(f32) 256:264 | pad] (768B rows)
T2_COLS = 128        # bf16: [h2 0:64 | a_s2(f32) 64:66 | pad] (256B)
ATT_COLS = 64        # f32 rows (256B): att1_local=[a_d(4)], att2_local=[a_d2(1)]

F32 = mybir.dt.float32
BF16 = mybir.dt.bfloat16
I16 = mybir.dt.int16

_EXEC_INFO = {}      # test.py reads this


# ---------------- CPU preprocessing ----------------

def _wrap_idx(vals):
    """int16 values -> [128, len/16] gather-index layout (16-wrap, x8 replicate)."""
    k = len(vals) // 16
    arr = np.asarray(vals, dtype=np.int16).reshape(k, 16).T  # [16, k]
    return np.tile(arr, (8, 1))  # [128, k]


def preprocess(edge_index):
    ei = np.asarray(edge_index)
    src = np.concatenate([ei[0], np.arange(N, dtype=ei.dtype)]).astype(np.int64)
    dst = np.concatenate([ei[1], np.arange(N, dtype=ei.dtype)]).astype(np.int64)
    order = np.argsort(dst, kind="stable")
    srcs = src[order].astype(np.int32)
    dsts = dst[order].astype(np.int32)

    # block boundaries: core c, block b covers nodes [c*NS + b*128, ...)
    lo_nodes = np.array([c * NS + b * 128 for c in range(NCORES) for b in range(NB)])
    hi_nodes = np.array(
        [min(c * NS + (b + 1) * 128, (c + 1) * NS) for c in range(NCORES) for b in range(NB)]
    )
    starts = np.searchsorted(dsts, lo_nodes)
    ends = np.searchsorted(dsts, hi_nodes)

    # per (c,b): split by src < SPLIT1 (same partition works for layer 2)
    n_lo = np.zeros((NCORES, NB), np.int64)
    n_hi = np.zeros((NCORES, NB), np.int64)
    segs = {}
    for c in range(NCORES):
        for b in range(NB):
            i = c * NB + b
            s, e = starts[i], ends[i]
            ss, dd = srcs[s:e], dsts[s:e]
            m = ss < SPLIT1
            segs[(c, b)] = (ss[m], dd[m], ss[~m], dd[~m])
            n_lo[c, b] = m.sum()
            n_hi[c, b] = (~m).sum()

    TL = [int(cdiv(int(n_lo[:, b].max()), P)) for b in range(NB)]
    TH = [int(cdiv(int(n_hi[:, b].max()), P)) for b in range(NB)]
    TL = [max(t, 1) for t in TL]
    TH = [max(t, 1) for t in TH]
    TB = [TL[b] + TH[b] for b in range(NB)]

    per_core = []
    for c in range(NCORES):
        idx1_cols, idx2_cols, idxa_cols, dloc_cols = [], [], [], []
        for b in range(NB):
            slo, dlo, shi, dhi = segs[(c, b)]
            npad_lo = TL[b] * P - len(slo)
            npad_hi = TH[b] * P - len(shi)
            blk_base = c * NS + b * 128

            s_all = np.concatenate([
                slo, np.zeros(npad_lo, np.int64),
                shi, np.full(npad_hi, SPLIT1, np.int64),
            ])
            d_all = np.concatenate([
                dlo, np.full(npad_lo, -1, np.int64),
                dhi, np.full(npad_hi, -1, np.int64),
            ])
            valid = d_all >= 0
            dloc = np.where(valid, d_all - blk_base, -1).astype(np.float32)

            # main1 idx: lo: src ; hi: src - SPLIT1
            i1 = s_all.copy()
            i1[TL[b] * P:] -= SPLIT1
            idx1_cols.append(np.concatenate([
                _wrap_idx(i1[: TL[b] * P]), _wrap_idx(i1[TL[b] * P:])], axis=1))

            # main2 idx: padded table2 row; lo: row ; hi: row - SPLIT2
            row2 = (s_all // NS) * NSP + (s_all % NS)
            i2 = row2.copy()
            i2[TL[b] * P:] -= SPLIT2
            assert i2.max() < 32768 and i2.min() >= 0
            idx2_cols.append(np.concatenate([
                _wrap_idx(i2[: TL[b] * P]), _wrap_idx(i2[TL[b] * P:])], axis=1))

            # att-dst idx: local shard row of dst (0 for pads)
            ia = np.where(valid, d_all - c * NS, 0)
            idxa_cols.append(_wrap_idx(ia))

            # dst_local tile-column layout [p, t]
            dloc_cols.append(dloc.reshape(TB[b], P).T.astype(ml_dtypes.bfloat16))

        per_core.append({
            "IDX1": np.concatenate(idx1_cols, axis=1),
            "IDX2": np.concatenate(idx2_cols, axis=1),
            "IDXA": np.concatenate(idxa_cols, axis=1),
            "DLOC": np.concatenate(dloc_cols, axis=1),
        })

    struct = {"TL": TL, "TH": TH, "TB": TB}
    return struct, per_core


# ---------------- device program ----------------

def build_program(struct):
    TL, TH, TB = struct["TL"], struct["TH"], struct["TB"]
    C1 = sum(TB) * 8          # idx cols for main (TL+TH per block, *8)
    CA = sum(TB) * 8
    CT = sum(TB)
    TMAX = max(TB)

    nc = bacc.Bacc("TRN2", target_bir_lowering=False, debug=False,
                   num_devices=NCORES)

    # inputs (xT/xmT are tile-major [nt][128ch][128node] for contiguous DMAs)
    NT1 = T1_ROWS // P  # 391
    xT = nc.dram_tensor("xT", [NT1, P, P], F32, kind="ExternalInput")
    xmT = nc.dram_tensor("xmT", [NB, P, P], F32, kind="ExternalInput")
    w1cat = nc.dram_tensor("w1cat", [P, 260], F32, kind="ExternalInput")
    vdst1 = nc.dram_tensor("vdst1", [P, 4], F32, kind="ExternalInput")
    w2cat = nc.dram_tensor("w2cat", [256, 66], BF16, kind="ExternalInput")
    wlin = nc.dram_tensor("wlin", [HID, OUT_CH], F32, kind="ExternalInput")
    b1rep = nc.dram_tensor("b1rep", [P, 256], F32, kind="ExternalInput")
    b2rep = nc.dram_tensor("b2rep", [P, HID], F32, kind="ExternalInput")
    blrep = nc.dram_tensor("blrep", [P, OUT_CH], F32, kind="ExternalInput")
    iota = nc.dram_tensor("iota", [P, 1, P], BF16, kind="ExternalInput")
    ident = nc.dram_tensor("ident", [P, P], F32, kind="ExternalInput")
    IDX1 = nc.dram_tensor("IDX1", [P, C1], I16, kind="ExternalInput")
    IDX2 = nc.dram_tensor("IDX2", [P, C1], I16, kind="ExternalInput")
    IDXA = nc.dram_tensor("IDXA", [P, CA], I16, kind="ExternalInput")
    DLOC = nc.dram_tensor("DLOC", [P, CT], BF16, kind="ExternalInput")

    out_sh = nc.dram_tensor("out", [NSP, OUT_CH], F32, kind="ExternalOutput")
    _ = None
    debug = int(os.environ.get("GAT_DEBUG", "0"))
    if debug:
        dbg = nc.dram_tensor("dbg", [NSP, 256], F32, kind="ExternalOutput")
    if debug >= 2:
        TB0 = TB[0]
        dbg_g = nc.dram_tensor("dbg_g", [P, TB0 * 320], F32, kind="ExternalOutput")
        dbg_ad = nc.dram_tensor("dbg_ad", [P, TB0 * 64], F32, kind="ExternalOutput")
        dbg_oh = nc.dram_tensor("dbg_oh", [P, TB0 * P], BF16, kind="ExternalOutput")
        dbg_e = nc.dram_tensor("dbg_e", [P, TB0 * 4], F32, kind="ExternalOutput")
        dbg_acc = nc.dram_tensor("dbg_acc", [P, 260], F32, kind="ExternalOutput")


    with tile.TileContext(nc) as tc:
        with (
            tc.tile_pool(name="dramp", bufs=1, space="DRAM") as dram,
            tc.tile_pool(name="const", bufs=1) as cst,
        ):
            # internal DRAM (pool tiles so Tile tracks cross-phase deps)
            t1 = dram.tile([T1_ROWS, T1_COLS], BF16, name="t1")
            att1 = dram.tile([NSP, ATT_COLS], F32, name="att1")
            att2 = dram.tile([NSP, ATT_COLS], F32, name="att2")
            t2s = dram.tile([NSP, T2_COLS], BF16, name="t2s")
            w1_sb = cst.tile([P, 260], F32)
            nc.sync.dma_start(out=w1_sb[:], in_=w1cat[:])
            vd1_sb = cst.tile([P, 4], F32)
            nc.sync.dma_start(out=vd1_sb[:], in_=vdst1[:])
            w2a_sb = cst.tile([P, 66], BF16)
            nc.sync.dma_start(out=w2a_sb[:], in_=w2cat[0:128, :])
            w2b_sb = cst.tile([P, 66], BF16)
            nc.sync.dma_start(out=w2b_sb[:], in_=w2cat[128:256, :])
            wl_sb = cst.tile([HID, OUT_CH], F32)
            nc.sync.dma_start(out=wl_sb[:], in_=wlin[:])
            b1_sb = cst.tile([P, 256], F32)
            nc.sync.dma_start(out=b1_sb[:], in_=b1rep[:])
            b2_sb = cst.tile([P, HID], F32)
            nc.sync.dma_start(out=b2_sb[:], in_=b2rep[:])
            bl_sb = cst.tile([P, OUT_CH], F32)
            nc.sync.dma_start(out=bl_sb[:], in_=blrep[:])
            io_sb = cst.tile([P, 1, P], BF16)
            nc.sync.dma_start(out=io_sb[:], in_=iota[:])
            id_sb = cst.tile([P, P], F32)
            nc.sync.dma_start(out=id_sb[:], in_=ident[:])
            x2T = cst.tile([P, 2, NSP], BF16)  # persistent layer-2 input (transposed)

            reps = int(os.environ.get("GAT_REPS", "1"))
            for _rep in range(reps):
                _build_phases(nc, tc, locals())
    nc.compile()
    return nc


def _build_phases(nc, tc, env):
    """One repetition of all compute phases (split out for GAT_REPS timing)."""
    TL, TH, TB = env["TL"], env["TH"], env["TB"]
    TMAX = env["TMAX"]
    xT, xmT = env["xT"], env["xmT"]
    w1_sb, vd1_sb = env["w1_sb"], env["vd1_sb"]
    w2a_sb, w2b_sb, wl_sb = env["w2a_sb"], env["w2b_sb"], env["wl_sb"]
    b1_sb, b2_sb, bl_sb = env["b1_sb"], env["b2_sb"], env["bl_sb"]
    io_sb, id_sb, x2T = env["io_sb"], env["id_sb"], env["x2T"]
    IDX1, IDX2, IDXA, DLOC = env["IDX1"], env["IDX2"], env["IDXA"], env["DLOC"]
    t1, att1, att2, t2s = (env["t1"], env["att1"], env["att2"], env["t2s"])
    t2f = env["dram"].tile([NCORES * NSP, T2_COLS], BF16, name="t2f",
                           addr_space="Shared")
    out_sh = env["out_sh"]
    debug = env["debug"]
    if debug:
        dbg = env["dbg"]
    if debug >= 2:
        dbg_g, dbg_ad, dbg_oh, dbg_e, dbg_acc = (
            env["dbg_g"], env["dbg_ad"], env["dbg_oh"], env["dbg_e"],
            env["dbg_acc"])
    NT1 = env["NT1"]

    if True:
        if True:
            # ---------- phase A: build table1 (replicated) ----------
            with (
                tc.tile_pool(name="pa", bufs=3) as pa,
                tc.tile_pool(name="pa_ps", bufs=2, space="PSUM") as pa_ps,
            ):
                for nt in range(NT1):
                    xt_t = pa.tile([P, P], F32, tag="xt")
                    nc.sync.dma_start(out=xt_t[:], in_=xT[nt, :, :])
                    ps = pa_ps.tile([P, 260], F32)
                    nc.tensor.matmul(ps[:], lhsT=xt_t[:], rhs=w1_sb[:],
                                     start=True, stop=True)
                    # h -> bf16, a_s stays f32 (written via bitcast cols)
                    rowh = pa.tile([P, 256], BF16, tag="rowh")
                    rowa = pa.tile([P, 4], F32, tag="rowa")
                    if nt % 2 == 0:
                        nc.vector.tensor_copy(out=rowh[:], in_=ps[:, 0:256])
                        nc.scalar.copy(rowa[:], ps[:, 256:260])
                    else:
                        nc.scalar.copy(rowh[:], ps[:, 0:256])
                        nc.vector.tensor_copy(out=rowa[:], in_=ps[:, 256:260])
                    nc.sync.dma_start(out=t1[nt * P:(nt + 1) * P, 0:256],
                                      in_=rowh[:])
                    nc.sync.dma_start(
                        out=t1[nt * P:(nt + 1) * P, 256:264].bitcast(F32),
                        in_=rowa[:])
                # phase A-mine: att1_local = x_mine @ Vdst1
                for b in range(NB):
                    xm_t = pa.tile([P, P], F32, tag="xt")
                    nc.sync.dma_start(out=xm_t[:], in_=xmT[b, :, :])
                    psm = pa_ps.tile([P, 260], F32)
                    nc.tensor.matmul(psm[:, 0:4], lhsT=xm_t[:], rhs=vd1_sb[:],
                                     start=True, stop=True)
                    rw4 = pa.tile([P, 4], F32, tag="rw4")
                    nc.vector.tensor_copy(out=rw4[:], in_=psm[:, 0:4])
                    nc.sync.dma_start(out=att1[b * P:(b + 1) * P, 0:4],
                                      in_=rw4[:])

            # ---------- layer 1 edge phase ----------
            def edge_layer(layer):
                if layer == 1:
                    FW = 260          # rhs width: 256 feat + 4 exp
                    NH = 4
                else:
                    FW = 65           # 64 feat + 1 exp
                    NH = 1
                with (
                    tc.tile_pool(name=f"eg{layer}", bufs=2) as eg,
                    tc.tile_pool(name=f"eg{layer}_ps", bufs=2, space="PSUM") as egp,
                    tc.tile_pool(name=f"ev{layer}", bufs=2) as ev,
                    tc.tile_pool(name=f"ev{layer}_ps", bufs=2, space="PSUM") as evp,
                ):
                    l1mode = os.environ.get("GAT_L1MODE", "full")
                    nb_lim = int(os.environ.get("GAT_NB", str(NB)))
                    off_m = 0
                    off_a = 0
                    off_t = 0
                    for b in range(min(NB, nb_lim) if layer == 1 else NB):
                        tl, th, tb = TL[b], TH[b], TB[b]
                        nv = 128 if b < NB - 1 else NS - 128 * (NB - 1)
                        # --- load idx/meta ---
                        ixm = eg.tile([P, TMAX * 8], I16, tag="ixm")
                        src_idx = IDX1 if layer == 1 else IDX2
                        nc.sync.dma_start(out=ixm[:, 0:tb * 8],
                                          in_=src_idx[:, off_m:off_m + tb * 8])
                        ixa = eg.tile([P, TMAX * 8], I16, tag="ixa")
                        nc.sync.dma_start(out=ixa[:, 0:tb * 8],
                                          in_=IDXA[:, off_a:off_a + tb * 8])
                        dlc = eg.tile([P, TMAX], BF16, tag="dlc")
                        nc.sync.dma_start(out=dlc[:, 0:tb],
                                          in_=DLOC[:, off_t:off_t + tb])
                        # --- gathers (chunked: >=1280 idx per dma_gather
                        # wedges the device; use <=1024 = 8 tiles) ---
                        def gat(out_t, c0, nt, src_ap, idx_t, ic0, elem):
                            done = 0
                            while done < nt:
                                k = min(8, nt - done)
                                nc.gpsimd.dma_gather(
                                    out_t[:, c0 + done:c0 + done + k, :],
                                    src_ap,
                                    idx_t[:, ic0 + done * 8:ic0 + (done + k) * 8],
                                    k * P, k * P, elem)
                                done += k

                        if layer == 1:
                            g = eg.tile([P, TMAX, 384], BF16, tag="g1")
                            gat(g, 0, tl, t1[:, 0:384], ixm, 0, 384)
                            gat(g, tl, th, t1[SPLIT1:T1_ROWS, 0:384],
                                ixm, tl * 8, 384)
                            ad = eg.tile([P, TMAX, 64], F32, tag="ad")
                            gat(ad, 0, tb, att1[:, 0:64], ixa, 0, 64)
                            a_s = g[:, 0:tb, 256:264].bitcast(F32)
                            a_d = ad[:, 0:tb, 0:4]
                            feat = lambda h: g[:, 0:tb, h * 64:(h + 1) * 64]
                        else:
                            g = eg.tile([P, TMAX, 128], BF16, tag="g2")
                            gat(g, 0, tl, t2f[0:SPLIT2, :], ixm, 0, 128)
                            gat(g, tl, th, t2f[SPLIT2:NCORES * NSP, :],
                                ixm, tl * 8, 128)
                            ad = eg.tile([P, TMAX, 64], F32, tag="ad")
                            gat(ad, 0, tb, att2[:, 0:64], ixa, 0, 64)
                            a_s = g[:, 0:tb, 64:66].bitcast(F32)
                            a_d = ad[:, 0:tb, 0:1]
                            feat = lambda h: g[:, 0:tb, 0:64]
                        if layer == 1 and l1mode == "gather":
                            cons = eg.tile([P, OUT_CH], F32, tag="cons")
                            nc.vector.tensor_copy(out=cons[:],
                                                  in_=g[:, 0, 0:OUT_CH])
                            nc.vector.tensor_tensor(out=cons[:], in0=cons[:],
                                                    in1=ad[:, 0, 0:OUT_CH],
                                                    op=mybir.AluOpType.add)
                            if b == 0:
                                nc.sync.dma_start(out=out_sh[0:P, :],
                                                  in_=cons[:])
                            off_m += tb * 8
                            off_a += tb * 8
                            off_t += tb
                            continue
                        # --- e = leaky(a_s + a_d); exp ---
                        s_t = eg.tile([P, TMAX, NH], F32, tag="s")
                        nc.vector.tensor_tensor(out=s_t[:, 0:tb, :], in0=a_s,
                                                in1=a_d, op=mybir.AluOpType.add)
                        s2_t = eg.tile([P, TMAX, NH], F32, tag="s2")
                        nc.vector.tensor_scalar_mul(s2_t[:, 0:tb, :],
                                                    s_t[:, 0:tb, :], NEG)
                        lk_t = eg.tile([P, TMAX, NH], F32, tag="lk")
                        nc.vector.tensor_tensor(out=lk_t[:, 0:tb, :],
                                                in0=s_t[:, 0:tb, :],
                                                in1=s2_t[:, 0:tb, :],
                                                op=mybir.AluOpType.max)
                        e_t = eg.tile([P, TMAX, NH], F32, tag="e")
                        nc.scalar.activation(e_t[:, 0:tb, :], lk_t[:, 0:tb, :],
                                             mybir.ActivationFunctionType.Exp)
                        # --- R = [feat*exp | exp] (bf16), onehot ---
                        r = eg.tile([P, TMAX, FW], BF16, tag="r")
                        for h in range(NH):
                            nc.vector.tensor_tensor(
                                out=r[:, 0:tb, h * 64:(h + 1) * 64],
                                in0=feat(h),
                                in1=e_t[:, 0:tb, h:h + 1].to_broadcast([P, tb, 64]),
                                op=mybir.AluOpType.mult)
                        nc.vector.tensor_copy(out=r[:, 0:tb, NH * 64:NH * 64 + NH],
                                              in_=e_t[:, 0:tb, :])
                        oh = eg.tile([P, TMAX, P], BF16, tag="oh")
                        nc.vector.tensor_tensor(
                            out=oh[:, 0:tb, :],
                            in0=io_sb[:].to_broadcast([P, tb, P]),
                            in1=dlc[:, 0:tb, None].to_broadcast([P, tb, P]),
                            op=mybir.AluOpType.is_equal)
                        # --- scatter matmuls ---
                        acc = egp.tile([P, FW], F32)
                        for t in range(tb):
                            nc.tensor.matmul(acc[:], lhsT=oh[:, t, :],
                                             rhs=r[:, t, :],
                                             start=(t == 0), stop=(t == tb - 1))
                        # --- eviction ---
                        NF = NH * 64
                        dn = ev.tile([P, NH], F32, tag="dn")
                        nc.vector.tensor_scalar_max(dn[:], acc[:, NF:NF + NH],
                                                    1e-30)
                        rc = ev.tile([P, NH], F32, tag="rc")
                        nc.vector.reciprocal(rc[:], dn[:])
                        xo = ev.tile([P, NF], F32, tag="xo")
                        for h in range(NH):
                            nc.vector.tensor_scalar_mul(
                                xo[:, h * 64:(h + 1) * 64],
                                acc[:, h * 64:(h + 1) * 64], rc[:, h:h + 1])
                        xb = ev.tile([P, NF], F32, tag="xb")
                        nc.vector.tensor_tensor(
                            out=xb[:], in0=xo[:],
                            in1=(b1_sb[:] if layer == 1 else b2_sb[:]),
                            op=mybir.AluOpType.add)
                        # ELU: exp(min(x,0)) + (max(x,0)-1)
                        mn = ev.tile([P, NF], F32, tag="mn")
                        nc.vector.tensor_scalar_min(mn[:], xb[:], 0.0)
                        ex = ev.tile([P, NF], F32, tag="ex")
                        nc.scalar.activation(ex[:], mn[:],
                                             mybir.ActivationFunctionType.Exp)
                        px = ev.tile([P, NF], F32, tag="px")
                        nc.vector.tensor_scalar(px[:], xb[:], 0.0, -1.0,
                                                mybir.AluOpType.max,
                                                mybir.AluOpType.add)
                        xe = ev.tile([P, NF], F32, tag="xe")
                        nc.vector.tensor_tensor(out=xe[:], in0=ex[:], in1=px[:],
                                                op=mybir.AluOpType.add)
                        if layer == 1 and debug:
                            nc.sync.dma_start(
                                out=dbg[b * P:b * P + nv, :], in_=xe[0:nv, :])
                        if layer == 1:
                            # transpose into persistent x2T (bf16)
                            for k in range(2):
                                tp = evp.tile([P, P], F32, tag="tp")
                                nc.tensor.transpose(tp[:], xe[:, k * P:(k + 1) * P],
                                                    id_sb[:])
                                nc.vector.tensor_copy(
                                    out=x2T[:, k, b * P:(b + 1) * P], in_=tp[:])
                            # fused table2-shard build for this block
                            ps2 = evp.tile([P, 66], F32, tag="ps2")
                            nc.tensor.matmul(ps2[:],
                                             lhsT=x2T[:, 0, b * P:(b + 1) * P],
                                             rhs=w2a_sb[:], start=True,
                                             stop=False)
                            nc.tensor.matmul(ps2[:],
                                             lhsT=x2T[:, 1, b * P:(b + 1) * P],
                                             rhs=w2b_sb[:], start=False,
                                             stop=True)
                            h2 = ev.tile([P, HID], BF16, tag="h2")
                            nc.vector.tensor_copy(out=h2[:], in_=ps2[:, 0:64])
                            av = ev.tile([P, 2], F32, tag="av")
                            nc.vector.tensor_copy(out=av[:], in_=ps2[:, 64:66])
                            nc.sync.dma_start(out=t2s[b * P:(b + 1) * P, 0:64],
                                              in_=h2[:])
                            nc.sync.dma_start(
                                out=t2s[b * P:(b + 1) * P, 64:66].bitcast(F32),
                                in_=av[:, 0:1])
                            nc.sync.dma_start(out=att2[b * P:(b + 1) * P, 0:1],
                                              in_=av[:, 1:2])
                        else:
                            # final linear: out = elu(x3) @ Wlin + blin
                            tp = evp.tile([P, P], F32, tag="tp")
                            nc.tensor.transpose(tp[0:64, 0:P], xe[:, 0:64],
                                                id_sb[:])
                            x3T = ev.tile([HID, P], F32, tag="x3T")
                            nc.vector.tensor_copy(out=x3T[:], in_=tp[0:64, 0:P])
                            ops = evp.tile([P, OUT_CH], F32, tag="ops")
                            nc.tensor.matmul(ops[:], lhsT=x3T[:], rhs=wl_sb[:],
                                             start=True, stop=True)
                            ob = ev.tile([P, OUT_CH], F32, tag="ob")
                            nc.vector.tensor_tensor(out=ob[:], in0=ops[:],
                                                    in1=bl_sb[:],
                                                    op=mybir.AluOpType.add)
                            nc.sync.dma_start(
                                out=out_sh[b * P:b * P + nv, :], in_=ob[0:nv, :])
                        off_m += tb * 8
                        off_a += tb * 8
                        off_t += tb

            phases = os.environ.get("GAT_PHASES", "full")
            if phases != "a":
                edge_layer(1)

            # ---------- AllGather (table2 shards built during L1 evict) ----------
            if phases in ("full", "a1tc"):
                nc.gpsimd.collective_compute(
                    "AllGather",
                    mybir.AluOpType.bypass,
                    replica_groups=[list(range(NCORES))],
                    ins=[t2s[:].opt()],
                    outs=[t2f[:].opt()],
                )

            # ---------- layer 2 edge phase + output ----------
            if phases == "full":
                edge_layer(2)


# ---------------- host orchestration ----------------

def _prep_weights(W1, att_src1, att_dst1, b1, W2, att_src2, att_dst2, b2,
                  Wlin, blin):
    W1 = np.asarray(W1, np.float32)
    vsrc1 = np.zeros((IN_CH, HEADS), np.float32)
    vdst1 = np.zeros((IN_CH, HEADS), np.float32)
    a_s1 = np.asarray(att_src1, np.float32)
    a_d1 = np.asarray(att_dst1, np.float32)
    for h in range(HEADS):
        vsrc1[:, h] = W1[:, h * HID:(h + 1) * HID] @ a_s1[h]
        vdst1[:, h] = W1[:, h * HID:(h + 1) * HID] @ a_d1[h]
    w1cat = np.concatenate([W1, vsrc1], axis=1)  # [128, 260]

    W2 = np.asarray(W2, np.float32)
    vsrc2 = W2 @ np.asarray(att_src2, np.float32)[0]
    vdst2 = W2 @ np.asarray(att_dst2, np.float32)[0]
    w2cat = np.concatenate([W2, vsrc2[:, None], vdst2[:, None]], axis=1)  # [256,66]

    import ml_dtypes

    return {
        "w1cat": w1cat,
        "vdst1": vdst1,
        "w2cat": w2cat.astype(ml_dtypes.bfloat16),
        "wlin": np.asarray(Wlin, np.float32),
        "b1rep": np.tile(np.asarray(b1, np.float32)[None, :], (P, 1)),
        "b2rep": np.tile(np.asarray(b2, np.float32)[None, :], (P, 1)),
        "blrep": np.tile(np.asarray(blin, np.float32)[None, :], (P, 1)),
        "iota": np.tile(np.arange(P).astype(ml_dtypes.bfloat16)[None, None, :],
                        (P, 1, 1)),
        "ident": np.eye(P, dtype=np.float32),
    }


_CACHE = {}


def time_kernel(nc, in_maps, iters=int(os.environ.get("GAT_ITERS", "5"))):
    """Steady-state device execution time: jit once, device-put inputs,
    time blocked executions (no host->device transfer in the loop)."""
    import time as _time
    import jax
    from jax.sharding import Mesh, PartitionSpec, NamedSharding
    from jax.experimental.shard_map import shard_map
    from concourse import bass2jax as b2j
    import concourse.mybir as mb

    b2j.install_neuronx_cc_hook()
    n_cores = len(in_maps)
    partition_name = (nc.partition_id_tensor.name
                      if nc.partition_id_tensor else None)
    in_names, out_names, out_avals, zero_outs = [], [], [], []
    for alloc in nc.m.functions[0].allocations:
        if not isinstance(alloc, mb.MemoryLocationSet):
            continue
        name = alloc.memorylocations[0].name
        if alloc.kind == "ExternalInput":
            if name != partition_name:
                in_names.append(name)
        elif alloc.kind == "ExternalOutput":
            out_avals.append(jax.core.ShapedArray(
                tuple(alloc.tensor_shape), mb.dt.np(alloc.dtype)))
            out_names.append(name)
            zero_outs.append(np.zeros(alloc.tensor_shape,
                                      mb.dt.np(alloc.dtype)))
    n_params = len(in_names)
    all_in_names = list(in_names) + list(out_names)
    if partition_name is not None:
        all_in_names.append(partition_name)

    def _body(*args):
        operands = list(args)
        if partition_name is not None:
            operands.append(b2j.partition_id_tensor())
        return tuple(b2j._bass_exec_p.bind(
            *operands,
            out_avals=tuple(out_avals),
            in_names=tuple(all_in_names),
            out_names=tuple(out_names),
            lowering_input_output_aliases=(),
            sim_require_finite=True,
            sim_require_nnan=True,
            nc=nc,
        ))

    devices = jax.devices()[:n_cores]
    mesh = Mesh(np.asarray(devices), ("core",))
    nouts = len(out_names)
    in_specs = (PartitionSpec("core"),) * (n_params + nouts)
    out_specs = (PartitionSpec("core"),) * nouts
    fn = jax.jit(shard_map(_body, mesh=mesh, in_specs=in_specs,
                           out_specs=out_specs, check_rep=False),
                 keep_unused=True)
    sh = NamedSharding(mesh, PartitionSpec("core"))
    concat_in = [
        jax.device_put(np.concatenate(
            [np.asarray(in_maps[c][nm]) for c in range(n_cores)], axis=0), sh)
        for nm in in_names
    ]
    concat_zero = [
        jax.device_put(np.zeros((n_cores * z.shape[0], *z.shape[1:]), z.dtype),
                       sh)
        for z in zero_outs
    ]
    outs = fn(*concat_in, *concat_zero)  # warmup / compile
    jax.block_until_ready(outs)
    pipe = int(os.environ.get("GAT_PIPE", "0"))
    if pipe:
        # submit `pipe` async executions, block once; device executions
        # serialize on the stream while tunnel latency overlaps.
        ts = []
        for _ in range(iters):
            t0 = _time.perf_counter()
            all_outs = [fn(*concat_in, *concat_zero) for _ in range(pipe)]
            jax.block_until_ready(all_outs)
            ts.append(_time.perf_counter() - t0)
        return min(ts) / pipe, ts
    ts = []
    for _ in range(iters):
        t0 = _time.perf_counter()
        outs = fn(*concat_in, *concat_zero)
        jax.block_until_ready(outs)
        ts.append(_time.perf_counter() - t0)
    return min(ts), ts


def kernel(x, edge_index, edge_attr, W1, att_src1, att_dst1, b1,
           W2, att_src2, att_dst2, b2, Wlin, blin):
    x = np.asarray(x, np.float32)
    struct, per_core = preprocess(edge_index)

    key = (tuple(struct["TL"]), tuple(struct["TH"]))
    if key not in _CACHE:
        _CACHE[key] = build_program(struct)
    nc = _CACHE[key]

    wts = _prep_weights(W1, att_src1, att_dst1, b1, W2, att_src2, att_dst2,
                        b2, Wlin, blin)
    xp = np.zeros((T1_ROWS, IN_CH), np.float32)
    xp[:N] = x
    # tile-major transposed: [nt, ch, node]
    xT = np.ascontiguousarray(xp.reshape(T1_ROWS // P, P, IN_CH).transpose(0, 2, 1))
    in_maps = []
    for c in range(NCORES):
        xmp = np.zeros((NSP, IN_CH), np.float32)
        xmp[:NS] = x[c * NS:(c + 1) * NS]
        xmT = np.ascontiguousarray(xmp.reshape(NB, P, IN_CH).transpose(0, 2, 1))
        m = {"xT": xT, "xmT": xmT}
        m.update(wts)
        m.update(per_core[c])
        in_maps.append(m)

    trace = bool(int(os.environ.get("GAT_TRACE", "0")))
    res = run_bass_kernel_spmd(nc, in_maps, core_ids=list(range(NCORES)),
                               trace=trace)
    _EXEC_INFO["exec_time_ns"] = res.exec_time_ns
    _EXEC_INFO["profile_json"] = res.profile_json

    if bool(int(os.environ.get("GAT_TIME", "0"))):
        best, ts = time_kernel(nc, in_maps)
        _EXEC_INFO["exec_time_ns"] = int(best * 1e9)
        _EXEC_INFO["times"] = ts

    if "dbg" in res.results[0]:
        _EXEC_INFO["dbg"] = np.concatenate(
            [res.results[c]["dbg"][0:NS] for c in range(NCORES)], axis=0)
    _EXEC_INFO["core0"] = res.results[0]

    out = np.concatenate([res.results[c]["out"][0:NS] for c in range(NCORES)],
                         axis=0)
    return out.astype(np.float32)
